# revision 5
# baseline (speedup 1.0000x reference)
"""Trainium2 Bass kernel for nn_MultiHeadedAttention (B=2, S=2048, D=1024, H=16).

Sharding: batch (2) x head-groups (4) -> 8 cores. Core c handles batch c//4,
heads [4*(c%4), 4*(c%4)+4). Per core everything runs in transposed layouts:

  phase 1: Q^T = Wq_s x_q^T, K^T = Wk_s x_k^T (features on partitions), and
           V in natural [seq, feat] layout with a ones-column appended per head.
  phase 2: per (head, q-block): S^T = K^T(tile)^T-contract Q^T  (k on
           partitions), exp on ScalarE with the 1/sqrt(dk) scale folded in
           (softmax without max-subtraction: |scores| <~ 8, safe in fp32),
           then PV via matmul with the ones-column producing the softmax
           denominators as row 64 of the accumulator.  Normalization uses
           reciprocal + gpsimd partition_broadcast + one vector multiply.
  phase 3: y^T partial = Wo_s^T x_attn^T; host sums the 4 partials per batch.

All matmuls run as float32r (full-rate fp32 mode on the PE at N>=256).
The tiny t-bias MLP ([B,1,1,1] -> [B,64]) is folded into the K projection
bias on the host during input sharding.
"""

import numpy as np

B, S, D, H, DK = 2, 2048, 1024, 16, 64
HPC = 4            # heads per core
DPC = HPC * DK     # 256 features per core
NCORES = 8

TRACE = False          # test harness sets True to capture an NTFF profile
LAST_EXEC_NS = None    # filled when TRACE
LAST_RESULTS = None

_BUILT = None


def _install_ntff_shim():
    """antenv.axon_hooks is absent in this image; recreate it so trace=True
    can ship NTFF profiles back through the axon tunnel."""
    import sys, types
    try:
        from antenv import axon_hooks  # noqa: F401
        return
    except ImportError:
        pass
    import antenv
    mod = types.ModuleType("antenv.axon_hooks")
    _hook = [None]
    mod.set_axon_ntff_profile_hook = lambda h: _hook.__setitem__(0, h)
    mod.get_axon_ntff_profile_hook = lambda: _hook[0]
    sys.modules["antenv.axon_hooks"] = mod
    antenv.axon_hooks = mod
    try:
        from trn_agent_boot.trn_boot import _ntff_profile_via_ctypes
        mod.set_axon_ntff_profile_hook(
            _ntff_profile_via_ctypes("/opt/axon/libaxon_pjrt.so"))
    except Exception:
        pass


def _build():
    """Build the per-core Bass graph (identical on all 8 cores)."""
    import concourse.tile as tile
    from concourse import mybir, bacc

    f32 = mybir.dt.float32
    f32r = mybir.dt.float32r

    nc = bacc.Bacc()

    xq_t = nc.dram_tensor("xq_t", [D, S], f32r, kind="ExternalInput")
    xk_t = nc.dram_tensor("xk_t", [D, S], f32r, kind="ExternalInput")
    xv_t = nc.dram_tensor("xv_t", [D, S], f32r, kind="ExternalInput")
    wq_t = nc.dram_tensor("wq_t", [D, DPC], f32r, kind="ExternalInput")
    wk_t = nc.dram_tensor("wk_t", [D, DPC], f32r, kind="ExternalInput")
    wv_t = nc.dram_tensor("wv_t", [D, DPC], f32r, kind="ExternalInput")
    wo_t = nc.dram_tensor("wo_t", [DPC, D], f32r, kind="ExternalInput")
    bq2 = nc.dram_tensor("bq2", [2, 128], f32, kind="ExternalInput")
    bk2 = nc.dram_tensor("bk2", [2, 128], f32, kind="ExternalInput")
    bv1 = nc.dram_tensor("bv1", [1, DPC], f32, kind="ExternalInput")
    bo8 = nc.dram_tensor("bo8", [8, 128], f32, kind="ExternalInput")
    y_t = nc.dram_tensor("y_t", [D, S], f32, kind="ExternalOutput")

    NQ = 4          # phase-1 seq quarters of 512
    QW = S // NQ
    NJ = 2          # attention q-blocks of 1024
    JW = S // NJ
    NE = D // 128   # 8 feature chunks
    NST = S // 128  # 16 seq tiles of 128 (k tiles)

    with tile.TileContext(nc) as tc:
        with tc.tile_pool(name="consts", bufs=1) as consts, \
             tc.tile_pool(name="persist", bufs=1) as persist:

            # ---- constants ----
            wq_sb = consts.tile([128, NE, DPC], f32r, tag="wq")
            wk_sb = consts.tile([128, NE, DPC], f32r, tag="wk")
            wv_sb = consts.tile([128, NE, DPC], f32r, tag="wv")
            for e in range(NE):
                nc.sync.dma_start(wq_sb[:, e, :], wq_t[e * 128:(e + 1) * 128, :])
                nc.sync.dma_start(wk_sb[:, e, :], wk_t[e * 128:(e + 1) * 128, :])
                nc.sync.dma_start(wv_sb[:, e, :], wv_t[e * 128:(e + 1) * 128, :])
            wo_sb = consts.tile([128, 2, D], f32r, tag="wo")
            for f in range(2):
                nc.sync.dma_start(wo_sb[:, f, :], wo_t[f * 128:(f + 1) * 128, :])
            bq_sb = consts.tile([128, 2], f32, tag="bq")
            bk_sb = consts.tile([128, 2], f32, tag="bk")
            for m in range(2):
                nc.sync.dma_start(bq_sb[:, m:m + 1], bq2[m].unsqueeze(1))
                nc.sync.dma_start(bk_sb[:, m:m + 1], bk2[m].unsqueeze(1))
            bo_sb = consts.tile([128, 8], f32, tag="bo")
            for o in range(8):
                nc.sync.dma_start(bo_sb[:, o:o + 1], bo8[o].unsqueeze(1))
            bv_row = consts.tile([1, DPC], f32, tag="bvr")
            nc.sync.dma_start(bv_row[0:1, :], bv1[0:1, :])
            bv_bc = consts.tile([128, HPC, DK], f32, tag="bvb")
            nc.gpsimd.partition_broadcast(
                bv_bc.rearrange("p h d -> p (h d)"), bv_row[0:1, :])

            # ---- persistent activations ----
            qt_sb = persist.tile([128, 2, S], f32r, tag="qt")   # [dpart, m, seq]
            kt_sb = persist.tile([128, 2, S], f32r, tag="kt")
            v_sb = persist.tile([128, NST, HPC, DK + 1], f32r, tag="v")
            ones1 = consts.tile([128, 1], f32, tag="ones1")
            nc.vector.memset(ones1[:, :], 1.0)
            nc.vector.tensor_copy(
                v_sb[:, :, :, DK:DK + 1].rearrange("p a b c -> p (a b c)"),
                ones1[:, 0:1].broadcast_to([128, NST * HPC]))

            # ================= phase 1: projections =================
            with tc.tile_pool(name="xin", bufs=2) as xin, \
                 tc.tile_pool(name="proj_ps", bufs=3, space="PSUM") as proj_ps, \
                 tc.tile_pool(name="v_ps", bufs=2, space="PSUM") as v_ps:
                for q in range(NQ):
                    qs = slice(q * QW, (q + 1) * QW)
                    xq_q = xin.tile([128, NE, QW], f32r, tag="xq")
                    xk_q = xin.tile([128, NE, QW], f32r, tag="xk")
                    xv_q = xin.tile([128, NE, QW], f32r, tag="xv")
                    for e in range(NE):
                        es = slice(e * 128, (e + 1) * 128)
                        nc.sync.dma_start(xq_q[:, e, :], xq_t[es, qs])
                        nc.sync.dma_start(xk_q[:, e, :], xk_t[es, qs])
                        nc.sync.dma_start(xv_q[:, e, :], xv_t[es, qs])
                    for m in range(2):
                        ms = slice(m * 128, (m + 1) * 128)
                        ps = proj_ps.tile([128, QW], f32, tag="proj")
                        for e in range(NE):
                            nc.tensor.matmul(ps[:, :], wq_sb[:, e, ms],
                                             xq_q[:, e, :],
                                             start=(e == 0), stop=(e == NE - 1))
                        nc.vector.tensor_scalar_add(
                            qt_sb[:, m, qs], ps[:, :], bq_sb[:, m:m + 1])
                        ps = proj_ps.tile([128, QW], f32, tag="proj")
                        for e in range(NE):
                            nc.tensor.matmul(ps[:, :], wk_sb[:, e, ms],
                                             xk_q[:, e, :],
                                             start=(e == 0), stop=(e == NE - 1))
                        nc.vector.tensor_scalar_add(
                            kt_sb[:, m, qs], ps[:, :], bk_sb[:, m:m + 1])
                    for st in range(4):
                        stg = q * 4 + st
                        ps = v_ps.tile([128, DPC], f32, tag="vps")
                        for e in range(NE):
                            nc.tensor.matmul(
                                ps[:, :],
                                xv_q[:, e, st * 128:(st + 1) * 128],
                                wv_sb[:, e, :],
                                start=(e == 0), stop=(e == NE - 1))
                        nc.vector.tensor_tensor(
                            out=v_sb[:, stg, :, 0:DK],
                            in0=ps.rearrange("p (h d) -> p h d", h=HPC),
                            in1=bv_bc[:, :, :],
                            op=mybir.AluOpType.add)

            # ================= phase 2+3: attention + out proj =================
            with tc.tile_pool(name="xattn", bufs=1) as xattn_pool, \
                 tc.tile_pool(name="psb", bufs=3) as p_pool, \
                 tc.tile_pool(name="rsb", bufs=2) as r_pool, \
                 tc.tile_pool(name="rbsb", bufs=2) as rb_pool, \
                 tc.tile_pool(name="ysb", bufs=3) as y_pool, \
                 tc.tile_pool(name="sc_ps", bufs=2, space="PSUM") as sc_ps, \
                 tc.tile_pool(name="o_ps", bufs=1, space="PSUM") as o_psp, \
                 tc.tile_pool(name="y_ps", bufs=2, space="PSUM") as y_psp:

                xa_sb = xattn_pool.tile([128, 2, S], f32r, tag="xa")

                for J in range(NJ):
                    Js = slice(J * JW, (J + 1) * JW)
                    for h in range(HPC):
                        pb = 64 * (h % 2)
                        hp = slice(pb, pb + DK)
                        m = h // 2
                        o_ps = o_psp.tile([DK + 1, JW], f32, tag="ops")
                        for i in range(NST):
                            ks = slice(i * 128, (i + 1) * 128)
                            s_ps = sc_ps.tile([128, JW], f32, tag="sc")
                            for half in range(2):
                                hs = slice(half * 512, half * 512 + 512)
                                jj = slice(J * JW + half * 512,
                                           J * JW + half * 512 + 512)
                                nc.tensor.matmul(s_ps[:, hs],
                                                 kt_sb[hp, m, ks],
                                                 qt_sb[hp, m, jj],
                                                 start=True, stop=True)
                            p_sb = p_pool.tile([128, JW], f32r, tag="p")
                            nc.scalar.activation(
                                p_sb[:, :], s_ps[:, :],
                                mybir.ActivationFunctionType.Exp, scale=0.125)
                            for half in range(2):
                                hs = slice(half * 512, half * 512 + 512)
                                nc.tensor.matmul(o_ps[:, hs],
                                                 v_sb[:, i, h, :],
                                                 p_sb[:, hs],
                                                 start=(i == 0),
                                                 stop=(i == NST - 1))
                        r_sb = r_pool.tile([1, JW], f32, tag="r")
                        nc.vector.reciprocal(r_sb[0:1, :], o_ps[DK:DK + 1, :])
                        rb_sb = rb_pool.tile([64, JW], f32, tag="rb")
                        nc.gpsimd.partition_broadcast(rb_sb[:, :], r_sb[0:1, :])
                        nc.vector.tensor_tensor(
                            out=xa_sb[hp, m, Js], in0=o_ps[0:DK, :],
                            in1=rb_sb[:, :], op=mybir.AluOpType.mult)
                    # out projection for this q-block
                    for o in range(8):
                        os_ = slice(o * 128, (o + 1) * 128)
                        y_sb = y_pool.tile([128, JW], f32, tag="y")
                        for half in range(2):
                            hs = slice(half * 512, half * 512 + 512)
                            jj = slice(J * JW + half * 512,
                                       J * JW + half * 512 + 512)
                            ps = y_psp.tile([128, 512], f32, tag="yps")
                            for f in range(2):
                                nc.tensor.matmul(ps[:, :],
                                                 wo_sb[:, f, os_],
                                                 xa_sb[:, f, jj],
                                                 start=(f == 0), stop=(f == 1))
                            nc.vector.tensor_scalar_add(
                                y_sb[:, hs], ps[:, :], bo_sb[:, o:o + 1])
                        nc.sync.dma_start(y_t[os_, Js], y_sb[:, :])

    nc.finalize()
    return nc


def _get_built():
    global _BUILT
    if _BUILT is None:
        _BUILT = _build()
    return _BUILT


def kernel(**inputs):
    global LAST_EXEC_NS, LAST_RESULTS
    from concourse import bass_utils

    inp = {k: np.ascontiguousarray(np.asarray(v), dtype=np.float32)
           for k, v in inputs.items()}

    # host: t-bias MLP, folded into the K-projection bias
    t = inp["t"].reshape(B)
    h1 = np.maximum(inp["tW1"][:, 0][None, :] * t[:, None] + inp["tb1"][None, :], 0.0)
    t_bias = h1 @ inp["tW2"].T + inp["tb2"][None, :]          # [B, DK]

    in_maps = []
    for c in range(NCORES):
        b, g = c // 4, c % 4
        sl = slice(g * DPC, (g + 1) * DPC)
        bo_full = inp["bo"] if g == 0 else np.zeros(D, np.float32)
        in_maps.append({
            "xq_t": np.ascontiguousarray(inp["query"][b].T),
            "xk_t": np.ascontiguousarray(inp["key"][b].T),
            "xv_t": np.ascontiguousarray(inp["value"][b].T),
            "wq_t": np.ascontiguousarray(inp["Wq"][sl, :].T),
            "wk_t": np.ascontiguousarray(inp["Wk"][sl, :].T),
            "wv_t": np.ascontiguousarray(inp["Wv"][sl, :].T),
            "wo_t": np.ascontiguousarray(inp["Wo"][:, sl].T),
            "bq2": inp["bq"][sl].reshape(2, 128).copy(),
            "bk2": (inp["bk"][sl] + np.tile(t_bias[b], HPC)).reshape(2, 128),
            "bv1": inp["bv"][sl].reshape(1, DPC).copy(),
            "bo8": bo_full.reshape(8, 128).copy(),
        })

    nc = _get_built()
    if TRACE:
        _install_ntff_shim()
    res = bass_utils.run_bass_kernel_spmd(
        nc, in_maps, core_ids=list(range(NCORES)), trace=TRACE)
    LAST_EXEC_NS = res.exec_time_ns
    LAST_RESULTS = res

    out = np.zeros((B, S, D), np.float32)
    for c in range(NCORES):
        out[c // 4] += res.results[c]["y_t"].T
    return out


# revision 6
# speedup vs baseline: 1.4915x; 1.4915x over previous
"""Trainium2 Bass kernel for nn_MultiHeadedAttention (B=2, S=2048, D=1024, H=16).

Sharding: batch (2) x head-groups (4) -> 8 cores. Core c handles batch c//4,
heads [4*(c%4), 4*(c%4)+4). Per core everything runs in transposed layouts:

  phase 1: Q^T = Wq_s x_q^T, K^T = Wk_s x_k^T (features on partitions), and
           V in natural [seq, feat] layout with a ones-column appended per head.
  phase 2: per (head, q-block): S^T = K^T(tile)^T-contract Q^T  (k on
           partitions), exp on ScalarE with the 1/sqrt(dk) scale folded in
           (softmax without max-subtraction: |scores| <~ 8, safe range),
           then PV via matmul with the ones-column producing the softmax
           denominators as row 64 of the accumulator.  Normalization uses
           reciprocal + gpsimd partition_broadcast + one vector multiply.
  phase 3: y^T partial = Wo_s^T x_attn^T; host sums the 4 partials per batch.

Matmul operands are bf16 (fp32 PSUM accumulation); fp32 everywhere else.
The tiny t-bias MLP ([B,1,1,1] -> [B,64]) is folded into the K projection
bias on the host during input sharding.
"""

import numpy as np

B, S, D, H, DK = 2, 2048, 1024, 16, 64
HPC = 4            # heads per core
DPC = HPC * DK     # 256 features per core
NCORES = 8

TRACE = False          # test harness sets True to capture an NTFF profile
LAST_EXEC_NS = None    # filled when TRACE
LAST_RESULTS = None

_BUILT = None


def _install_ntff_shim():
    """antenv.axon_hooks is absent in this image; recreate it so trace=True
    can ship NTFF profiles back through the axon tunnel."""
    import sys, types
    try:
        from antenv import axon_hooks  # noqa: F401
        return
    except ImportError:
        pass
    import antenv
    mod = types.ModuleType("antenv.axon_hooks")
    _hook = [None]
    mod.set_axon_ntff_profile_hook = lambda h: _hook.__setitem__(0, h)
    mod.get_axon_ntff_profile_hook = lambda: _hook[0]
    sys.modules["antenv.axon_hooks"] = mod
    antenv.axon_hooks = mod
    try:
        from trn_agent_boot.trn_boot import _ntff_profile_via_ctypes
        mod.set_axon_ntff_profile_hook(
            _ntff_profile_via_ctypes("/opt/axon/libaxon_pjrt.so"))
    except Exception:
        pass


def _build():
    """Build the per-core Bass graph (identical on all 8 cores)."""
    import concourse.tile as tile
    from concourse import mybir, bacc

    f32 = mybir.dt.float32
    bf16 = mybir.dt.bfloat16

    nc = bacc.Bacc()

    xq_t = nc.dram_tensor("xq_t", [D, S], bf16, kind="ExternalInput")
    xk_t = nc.dram_tensor("xk_t", [D, S], bf16, kind="ExternalInput")
    xv_t = nc.dram_tensor("xv_t", [D, S], bf16, kind="ExternalInput")
    wq_t = nc.dram_tensor("wq_t", [D, DPC], bf16, kind="ExternalInput")
    wk_t = nc.dram_tensor("wk_t", [D, DPC], bf16, kind="ExternalInput")
    wv_t = nc.dram_tensor("wv_t", [D, DPC], bf16, kind="ExternalInput")
    wo_t = nc.dram_tensor("wo_t", [DPC, D], bf16, kind="ExternalInput")
    bq2 = nc.dram_tensor("bq2", [2, 128], f32, kind="ExternalInput")
    bk2 = nc.dram_tensor("bk2", [2, 128], f32, kind="ExternalInput")
    bv1 = nc.dram_tensor("bv1", [1, DPC], f32, kind="ExternalInput")
    bo8 = nc.dram_tensor("bo8", [8, 128], f32, kind="ExternalInput")
    y_t = nc.dram_tensor("y_t", [D, S], f32, kind="ExternalOutput")

    NB = 2          # phase-1 seq blocks of 1024
    BW = S // NB
    NJ = 2          # attention q-blocks of 1024
    JW = S // NJ
    NE = D // 128   # 8 feature chunks
    NST = S // 128  # 16 seq tiles of 128 (k tiles)

    with tile.TileContext(nc) as tc:
        with tc.tile_pool(name="consts", bufs=1) as consts, \
             tc.tile_pool(name="persist", bufs=1) as persist:

            # ---- constants ----
            wq_sb = consts.tile([128, NE, DPC], bf16, tag="wq")
            wk_sb = consts.tile([128, NE, DPC], bf16, tag="wk")
            wv_sb = consts.tile([128, NE, DPC], bf16, tag="wv")
            for e in range(NE):
                nc.sync.dma_start(wq_sb[:, e, :], wq_t[e * 128:(e + 1) * 128, :])
                nc.sync.dma_start(wk_sb[:, e, :], wk_t[e * 128:(e + 1) * 128, :])
                nc.sync.dma_start(wv_sb[:, e, :], wv_t[e * 128:(e + 1) * 128, :])
            wo_sb = consts.tile([128, 2, D], bf16, tag="wo")
            for f in range(2):
                nc.sync.dma_start(wo_sb[:, f, :], wo_t[f * 128:(f + 1) * 128, :])
            bq_sb = consts.tile([128, 2], f32, tag="bq")
            bk_sb = consts.tile([128, 2], f32, tag="bk")
            for m in range(2):
                nc.sync.dma_start(bq_sb[:, m:m + 1], bq2[m].unsqueeze(1))
                nc.sync.dma_start(bk_sb[:, m:m + 1], bk2[m].unsqueeze(1))
            bo_sb = consts.tile([128, 8], f32, tag="bo")
            for o in range(8):
                nc.sync.dma_start(bo_sb[:, o:o + 1], bo8[o].unsqueeze(1))
            bv_row = consts.tile([1, DPC], f32, tag="bvr")
            nc.sync.dma_start(bv_row[0:1, :], bv1[0:1, :])
            bv_bc = consts.tile([128, HPC, DK], f32, tag="bvb")
            nc.gpsimd.partition_broadcast(
                bv_bc.rearrange("p h d -> p (h d)"), bv_row[0:1, :])

            # ---- persistent activations ----
            qt_sb = persist.tile([128, 2, S], bf16, tag="qt")   # [dpart, m, seq]
            kt_sb = persist.tile([128, 2, S], bf16, tag="kt")
            v_sb = persist.tile([128, NST, HPC, DK + 1], bf16, tag="v")
            ones1 = consts.tile([128, 1], f32, tag="ones1")
            nc.vector.memset(ones1[:, :], 1.0)
            nc.vector.tensor_copy(
                v_sb[:, :, :, DK:DK + 1].rearrange("p a b c -> p (a b c)"),
                ones1[:, 0:1].broadcast_to([128, NST * HPC]))

            # ================= phase 1: projections =================
            with tc.tile_pool(name="xin", bufs=2) as xin, \
                 tc.tile_pool(name="proj_ps", bufs=2, space="PSUM") as proj_ps, \
                 tc.tile_pool(name="v_ps", bufs=2, space="PSUM") as v_ps:
                for q in range(NB):
                    qs = slice(q * BW, (q + 1) * BW)
                    xq_q = xin.tile([128, NE, BW], bf16, tag="xq")
                    xk_q = xin.tile([128, NE, BW], bf16, tag="xk")
                    xv_q = xin.tile([128, NE, BW], bf16, tag="xv")
                    for e in range(NE):
                        es = slice(e * 128, (e + 1) * 128)
                        nc.sync.dma_start(xq_q[:, e, :], xq_t[es, qs])
                        nc.sync.dma_start(xk_q[:, e, :], xk_t[es, qs])
                        nc.sync.dma_start(xv_q[:, e, :], xv_t[es, qs])
                    for m in range(2):
                        ms = slice(m * 128, (m + 1) * 128)
                        ps = proj_ps.tile([128, BW], f32, tag="proj")
                        for e in range(NE):
                            for hf in range(2):
                                hs = slice(hf * 512, hf * 512 + 512)
                                nc.tensor.matmul(ps[:, hs], wq_sb[:, e, ms],
                                                 xq_q[:, e, hs],
                                                 start=(e == 0),
                                                 stop=(e == NE - 1))
                        nc.vector.tensor_scalar_add(
                            qt_sb[:, m, qs], ps[:, :], bq_sb[:, m:m + 1])
                        ps = proj_ps.tile([128, BW], f32, tag="proj")
                        for e in range(NE):
                            for hf in range(2):
                                hs = slice(hf * 512, hf * 512 + 512)
                                nc.tensor.matmul(ps[:, hs], wk_sb[:, e, ms],
                                                 xk_q[:, e, hs],
                                                 start=(e == 0),
                                                 stop=(e == NE - 1))
                        nc.vector.tensor_scalar_add(
                            kt_sb[:, m, qs], ps[:, :], bk_sb[:, m:m + 1])
                    for st in range(8):
                        stg = q * 8 + st
                        ps = v_ps.tile([128, DPC], f32, tag="vps")
                        for e in range(NE):
                            nc.tensor.matmul(
                                ps[:, :],
                                xv_q[:, e, st * 128:(st + 1) * 128],
                                wv_sb[:, e, :],
                                start=(e == 0), stop=(e == NE - 1))
                        nc.vector.tensor_tensor(
                            out=v_sb[:, stg, :, 0:DK],
                            in0=ps.rearrange("p (h d) -> p h d", h=HPC),
                            in1=bv_bc[:, :, :],
                            op=mybir.AluOpType.add)

            # ================= phase 2+3: attention + out proj =================
            with tc.tile_pool(name="xattn", bufs=1) as xattn_pool, \
                 tc.tile_pool(name="psb", bufs=3) as p_pool, \
                 tc.tile_pool(name="rsb", bufs=2) as r_pool, \
                 tc.tile_pool(name="rbsb", bufs=2) as rb_pool, \
                 tc.tile_pool(name="ysb", bufs=3) as y_pool, \
                 tc.tile_pool(name="sc_ps", bufs=2, space="PSUM") as sc_ps, \
                 tc.tile_pool(name="o_ps", bufs=2, space="PSUM") as o_psp:

                xa_sb = xattn_pool.tile([128, 2, S], bf16, tag="xa")

                for J in range(NJ):
                    Js = slice(J * JW, (J + 1) * JW)
                    for h in range(HPC):
                        pb = 64 * (h % 2)
                        hp = slice(pb, pb + DK)
                        m = h // 2
                        o_ps = o_psp.tile([DK + 1, JW], f32, tag="ops")
                        for i in range(NST):
                            ks = slice(i * 128, (i + 1) * 128)
                            s_ps = sc_ps.tile([128, JW], f32, tag="sc")
                            for half in range(2):
                                hs = slice(half * 512, half * 512 + 512)
                                jj = slice(J * JW + half * 512,
                                           J * JW + half * 512 + 512)
                                nc.tensor.matmul(s_ps[:, hs],
                                                 kt_sb[hp, m, ks],
                                                 qt_sb[hp, m, jj],
                                                 start=True, stop=True)
                            p_sb = p_pool.tile([128, JW], bf16, tag="p")
                            nc.scalar.activation(
                                p_sb[:, :], s_ps[:, :],
                                mybir.ActivationFunctionType.Exp, scale=0.125)
                            for half in range(2):
                                hs = slice(half * 512, half * 512 + 512)
                                nc.tensor.matmul(o_ps[:, hs],
                                                 v_sb[:, i, h, :],
                                                 p_sb[:, hs],
                                                 start=(i == 0),
                                                 stop=(i == NST - 1))
                        r_sb = r_pool.tile([1, JW], f32, tag="r")
                        nc.vector.reciprocal(r_sb[0:1, :], o_ps[DK:DK + 1, :])
                        rb_sb = rb_pool.tile([64, JW], f32, tag="rb")
                        nc.gpsimd.partition_broadcast(rb_sb[:, :], r_sb[0:1, :])
                        nc.vector.tensor_tensor(
                            out=xa_sb[hp, m, Js], in0=o_ps[0:DK, :],
                            in1=rb_sb[:, :], op=mybir.AluOpType.mult)
                    # out projection for this q-block (psum shares slots with sc)
                    for o in range(8):
                        os_ = slice(o * 128, (o + 1) * 128)
                        y_sb = y_pool.tile([128, JW], f32, tag="y")
                        for half in range(2):
                            hs = slice(half * 512, half * 512 + 512)
                            jj = slice(J * JW + half * 512,
                                       J * JW + half * 512 + 512)
                            ps = sc_ps.tile([128, 512], f32, tag="sc")
                            for f in range(2):
                                nc.tensor.matmul(ps[:, :],
                                                 wo_sb[:, f, os_],
                                                 xa_sb[:, f, jj],
                                                 start=(f == 0), stop=(f == 1))
                            nc.vector.tensor_scalar_add(
                                y_sb[:, hs], ps[:, :], bo_sb[:, o:o + 1])
                        nc.sync.dma_start(y_t[os_, Js], y_sb[:, :])

    nc.finalize()
    return nc


def _get_built():
    global _BUILT
    if _BUILT is None:
        _BUILT = _build()
    return _BUILT


def kernel(**inputs):
    global LAST_EXEC_NS, LAST_RESULTS
    import ml_dtypes
    from concourse import bass_utils

    bf16 = ml_dtypes.bfloat16
    inp = {k: np.ascontiguousarray(np.asarray(v), dtype=np.float32)
           for k, v in inputs.items()}

    # host: t-bias MLP, folded into the K-projection bias
    t = inp["t"].reshape(B)
    h1 = np.maximum(inp["tW1"][:, 0][None, :] * t[:, None] + inp["tb1"][None, :], 0.0)
    t_bias = h1 @ inp["tW2"].T + inp["tb2"][None, :]          # [B, DK]

    in_maps = []
    for c in range(NCORES):
        b, g = c // 4, c % 4
        sl = slice(g * DPC, (g + 1) * DPC)
        bo_full = inp["bo"] if g == 0 else np.zeros(D, np.float32)
        in_maps.append({
            "xq_t": np.ascontiguousarray(inp["query"][b].T.astype(bf16)),
            "xk_t": np.ascontiguousarray(inp["key"][b].T.astype(bf16)),
            "xv_t": np.ascontiguousarray(inp["value"][b].T.astype(bf16)),
            "wq_t": np.ascontiguousarray(inp["Wq"][sl, :].T.astype(bf16)),
            "wk_t": np.ascontiguousarray(inp["Wk"][sl, :].T.astype(bf16)),
            "wv_t": np.ascontiguousarray(inp["Wv"][sl, :].T.astype(bf16)),
            "wo_t": np.ascontiguousarray(inp["Wo"][:, sl].T.astype(bf16)),
            "bq2": inp["bq"][sl].reshape(2, 128).copy(),
            "bk2": (inp["bk"][sl] + np.tile(t_bias[b], HPC)).reshape(2, 128),
            "bv1": inp["bv"][sl].reshape(1, DPC).copy(),
            "bo8": bo_full.reshape(8, 128).copy(),
        })

    nc = _get_built()
    if TRACE:
        _install_ntff_shim()
    res = bass_utils.run_bass_kernel_spmd(
        nc, in_maps, core_ids=list(range(NCORES)), trace=TRACE)
    LAST_EXEC_NS = res.exec_time_ns
    LAST_RESULTS = res

    out = np.zeros((B, S, D), np.float32)
    for c in range(NCORES):
        out[c // 4] += res.results[c]["y_t"].T
    return out
